# revision 6
# baseline (speedup 1.0000x reference)
"""Trainium2 Bass kernel for the dense-MLP Bayesian log-joint problem.

Computes, for fixed MLP weights (1024 -> 2048 -> 2048 -> 10):
    h1 = relu(X @ W1.T + b1); h2 = relu(h1 @ W2.T + b2)
    logits = h2 @ W3.T + b3
    out = sum_i log_softmax(logits)[i, Y[i]] + log MVN(0, 100 I)(params)

Data-parallel over 8 NeuronCores: each core gets 2048 rows of X/Y and a
replicated copy of the (small) weights, computes its partial
log-likelihood on-device; the host sums the partials and adds the
closed-form Gaussian prior plus the b3 term of the one-hot pick.

Device kernel (fp8e4m3 DoubleRow matmuls, fp32 PSUM accumulation):
- All weights and the per-core X shard are SBUF-resident; the repeat
  loop used for timing contains no DMA except the final 4-byte result.
- Layers 1/2: per 128-row output tile, DoubleRow matmuls accumulate
  into a [128, 2048] PSUM tile (4 banks); the Activation engine
  evacuates with fused relu(s*psum + b) -> fp8.
- Layer 3 + log-softmax epilogue in 2 column chunks, each a short
  pipeline: Exp(s3*psum + b3) -> bf16 on the Activation engine, one-hot
  pick via a single fused DVE scalar_tensor_tensor with accum_out, a
  bf16 ones-matmul for sum-exp, and Ln with accum_out. The chunk q-1
  sum-exp/Ln work is emitted after chunk q's matmuls so the PE never
  idles waiting on the Activation engine.

Measured facts from this hardware (8 axon TRN2 cores, For_i loop,
paired trip-count differencing, same-process A/B):
- fp8 DoubleRow matmul streams at ~4/3 PE cycles per output column
  (measured on 512-col and 256-col MMs; plain fp8/bf16 ~1.14 cyc/col).
  LDWEIGHTS reuse (same lhsT for all MMs) and pair-interleaved rhs
  layouts change nothing; this rate is intrinsic, so the work here has
  a ~220 us pure-PE floor and the kernel runs at ~98% of it. 256-col
  MMs are ~1-4% faster per column than 512-col (mw=256 default);
  128-col is much slower (per-MM overhead). NOTE: start=True clears
  has_written for the whole 2KB PSUM bank, so sub-bank column groups
  must use the bank_lead logic below. Splitting PSUM evacuation across
  ACT+DVE engines or halving PSUM tiles made layers SLOWER (do not
  revisit without re-measuring).
- InstTensorTensorReduce crashes on hardware (works in CoreSim); use
  scalar_tensor_tensor (InstTensorScalarPtr) with accum_out instead.
- The For_i all-engine barrier costs ~0 ns/rep, but drains the pipeline
  between iterations; stage "fullxN" (N complete evaluations per
  iteration; test.py times fullx4) recovers the epilogue/layer-1
  overlap (~4-7 us/eval).
- Per-process clock drift is ~±4%: only trust same-process A/B deltas.

~217-225 us per evaluation (vs 232 us for the previous kernel in the
same processes, ~254 us baseline).
"""

import math

import numpy as np
import ml_dtypes

N = 16384
D = 1024
H = 2048
C = 10
CP = 16
N_CORES = 8
NL = N // N_CORES
PRIOR_VAR = 100.0

BF16 = ml_dtypes.bfloat16
E4M3 = ml_dtypes.float8_e4m3

SX = 16.0
SW = 128.0
SH = 16.0

KD = D // 128   # 8
KH = H // 128   # 16
NQ = 4          # epilogue column quarters
QW = NL // NQ   # 512

_compiled = {}


def _emit(tc, ctx, aps, repeat, stage="full", hw_loop=False, lmode="full", nq=NQ, mw=256):
    import contextlib

    from concourse import mybir

    nc = tc.nc
    f32 = mybir.dt.float32
    bf16 = mybir.dt.bfloat16
    fp8 = mybir.dt.float8e4
    AF = mybir.ActivationFunctionType
    ALU = mybir.AluOpType
    perf_mode = mybir.MatmulPerfMode.DoubleRow
    s12 = SH / (SX * SW)          # PSUM -> h rescale, layers 1 and 2
    s3 = 1.0 / (SH * SW)          # PSUM -> logits rescale, layer 3

    xt, w1, w2, w3 = aps["xt"], aps["w1"], aps["w2"], aps["w3"]
    b1, b1d, b2, b2d = aps["b1"], aps["b1d"], aps["b2"], aps["b2d"]
    b3, oh, out = aps["b3"], aps["oh"], aps["out"]

    consts = ctx.enter_context(tc.tile_pool(name="consts", bufs=1))
    acts = ctx.enter_context(tc.tile_pool(name="acts", bufs=1))
    tmpp = ctx.enter_context(tc.tile_pool(name="tmpp", bufs=2))
    if lmode == "half":
        psum = ctx.enter_context(tc.tile_pool(name="psum", bufs=4, space="PSUM"))
    else:
        psum = ctx.enter_context(tc.tile_pool(name="psum", bufs=2, space="PSUM"))
    epil = ctx.enter_context(tc.tile_pool(name="epil", bufs=2))

    # ---- SBUF-resident constants (loaded once, before the loop)
    xt_sb = consts.tile([128, KD, NL], fp8, name="xt_sb")
    for kd in range(KD):
        nc.sync.dma_start(out=xt_sb[:, kd, :], in_=xt[:, kd, :])
    w1_sb = consts.tile([128, KH, KD, 128], fp8, name="w1_sb")
    for m in range(KH):
        nc.sync.dma_start(out=w1_sb[:, m], in_=w1[m])
    w2_sb = consts.tile([128, KH, KH, 128], fp8, name="w2_sb")
    for m in range(KH):
        nc.sync.dma_start(out=w2_sb[:, m], in_=w2[m])
    w3_sb = consts.tile([128, KH, CP], fp8, name="w3_sb")
    nc.sync.dma_start(out=w3_sb, in_=w3)
    b1_sb = consts.tile([128, KH], f32, name="b1_sb")
    nc.sync.dma_start(out=b1_sb, in_=b1)
    b1d_sb = consts.tile([128, KH], f32, name="b1d_sb")
    nc.sync.dma_start(out=b1d_sb, in_=b1d)
    b2_sb = consts.tile([128, KH], f32, name="b2_sb")
    nc.sync.dma_start(out=b2_sb, in_=b2)
    b2d_sb = consts.tile([128, KH], f32, name="b2d_sb")
    nc.sync.dma_start(out=b2d_sb, in_=b2d)
    b3_sb = consts.tile([C, 1], f32, name="b3_sb")
    nc.sync.dma_start(out=b3_sb, in_=b3)
    oh_sb = consts.tile([C, NL], f32, name="oh_sb")
    nc.sync.dma_start(out=oh_sb, in_=oh)
    ones_bf = consts.tile([C, 1], bf16, name="ones_bf")
    nc.vector.memset(ones_bf, 1.0)
    # pt matmul lhsT: carries the s3 logits rescale of the one-hot pick
    ones_f = consts.tile([C, 1], f32, name="ones_f")
    nc.vector.memset(ones_f, s3)
    res_sb = consts.tile([1, 1], f32, name="res_sb")

    h1_sb = acts.tile([128, KH, NL], fp8, name="h1_sb")
    h2_sb = acts.tile([128, KH, NL], fp8, name="h2_sb")

    HB = NL // 2  # column half width for layers 1/2

    def dve_relu(ps_cols, out_cols, b_dve, m):
        """relu(s12*ps + b) on the Vector engine: (ps + b/s12), max*s12."""
        t = tmpp.tile([128, HB], f32, name="t", tag="t")
        nc.vector.tensor_scalar(
            out=t, in0=ps_cols, scalar1=b_dve[:, m:m + 1], scalar2=None,
            op0=ALU.add)
        nc.vector.tensor_scalar(
            out=out_cols, in0=t,
            scalar1=0.0, scalar2=s12, op0=ALU.max, op1=ALU.mult)

    def layer(w_sb, rhs_sb, kt, h_out, b_act, b_dve):
        """h_out[:, m, :] = relu(s12 * (w.T @ rhs) + bias) for all m."""
        for m in range(KH):
            if lmode == "half":
                pss = [psum.tile([128, HB], f32, name=f"ps{h}", tag="mm")
                       for h in range(2)]
            else:
                ps_full = psum.tile([128, NL], f32, name="ps", tag="mm")
            for k in range(0, kt, 2):
                for ns in range(NL // mw):
                    if lmode == "half":
                        ps = pss[ns * mw // HB]
                        col = (ns * mw) % HB
                        dst = ps[:, col:col + mw]
                    else:
                        dst = ps_full[:, ns * mw:(ns + 1) * mw]
                    # start=True clears has_written for the WHOLE 2KB
                    # bank, so when mw < 512 only the bank-aligned group
                    # may issue it; the unaligned neighbor's first MM
                    # relies on overwrite-where-bit-clear semantics.
                    bank_lead = (ns * mw) % 512 == 0
                    nc.tensor.matmul(
                        dst,
                        lhsT=w_sb[:, m, k:k + 2, :],
                        rhs=rhs_sb[:, k:k + 2, ns * mw:(ns + 1) * mw],
                        start=(k == 0) and bank_lead,
                        stop=(k + 2 >= kt),
                        skip_group_check=not bank_lead,
                        perf_mode=perf_mode,
                    )
            if lmode == "half":
                nc.scalar.activation(
                    out=h_out[:, m, 0:HB], in_=pss[0],
                    func=AF.Relu, bias=b_act[:, m:m + 1], scale=s12,
                )
                dve_relu(pss[1], h_out[:, m, HB:NL], b_dve, m)
            elif lmode == "fullsplit":
                nc.scalar.activation(
                    out=h_out[:, m, 0:HB], in_=ps_full[:, 0:HB],
                    func=AF.Relu, bias=b_act[:, m:m + 1], scale=s12,
                )
                dve_relu(ps_full[:, HB:NL], h_out[:, m, HB:NL], b_dve, m)
            else:  # "full": v1-style single Activation evacuation
                nc.scalar.activation(
                    out=h_out[:, m, :], in_=ps_full,
                    func=AF.Relu, bias=b_act[:, m:m + 1], scale=s12,
                )

    def finish_early(src):
        nc.vector.tensor_reduce(
            out=res_sb, in_=src[0:1, 0, 0:128],
            axis=mybir.AxisListType.X, op=ALU.add)

    xt_int = None
    if stage == "mmint":
        # pair-interleaved copy of xt, built once before the loop
        xt_int = acts.tile([128, KD // 2, NL, 2], fp8, name="xt_int")
        for j in range(KD // 2):
            for p in range(2):
                nc.vector.tensor_copy(
                    out=xt_int[:, j, :, p], in_=xt_sb[:, 2 * j + p, :])

    body_reps = 1
    if stage.startswith("fullx"):
        # multiple complete evaluations per loop iteration: one eval's
        # epilogue overlaps the next eval's layer-1 matmuls (the For_i
        # barrier otherwise drains the pipeline between evaluations)
        body_reps = int(stage[5:])
        stage = "full"

    if hw_loop and repeat > 1:
        reps = range(body_reps)
        loop_cm = tc.For_i(0, repeat, 1,
                           hint_engines=(mybir.EngineType.PE,))
    else:
        reps = range(repeat * body_reps)
        loop_cm = contextlib.nullcontext()

    with loop_cm:
     for _rep in reps:
        if stage == "empty":
            nc.vector.memset(res_sb, 0.0)
            continue

        # ---- engine-rate probes (timing only, garbage output)
        if stage in ("mm1", "mm2"):
            kt = KD if stage == "mm1" else KH
            nm = 16 if stage == "mm1" else 16
            for m in range(nm):
                ps_full = psum.tile([128, NL], f32, name="ps", tag="mm")
                for k in range(0, kt, 2):
                    for ns in range(4):
                        nc.tensor.matmul(
                            ps_full[:, ns * 512:(ns + 1) * 512],
                            lhsT=w2_sb[:, m, k:k + 2, :],
                            rhs=xt_sb[:, (k % KD):(k % KD) + 2,
                                      ns * 512:(ns + 1) * 512],
                            start=(k == 0), stop=(k + 2 >= kt),
                            perf_mode=perf_mode,
                        )
                if m == nm - 1:
                    nc.scalar.activation(
                        out=h1_sb[:, 0, :], in_=ps_full, func=AF.Relu,
                        bias=b1_sb[:, 0:1], scale=s12)
            finish_early(h1_sb)
            continue
        if stage in ("mmsame", "mmhalf"):
            # mmsame: 256 L1-shaped MMs, all with the same lhsT tile.
            # mmhalf: 512 MMs of 256 output columns each (same total work).
            ncols = 256 if stage == "mmhalf" else 512
            nss = NL // ncols
            for m in range(16):
                ps_full = psum.tile([128, NL], f32, name="ps", tag="mm")
                for k in range(0, KD, 2):
                    for ns in range(nss):
                        lk = 0 if stage == "mmsame" else k
                        nc.tensor.matmul(
                            ps_full[:, ns * ncols:(ns + 1) * ncols],
                            lhsT=w2_sb[:, m if stage == "mmhalf" else 0,
                                       lk:lk + 2, :],
                            rhs=xt_sb[:, k:k + 2, ns * ncols:(ns + 1) * ncols],
                            start=(k == 0), stop=(k + 2 >= KD),
                            perf_mode=perf_mode,
                        )
                if m == 15:
                    nc.scalar.activation(
                        out=h1_sb[:, 0, :], in_=ps_full, func=AF.Relu,
                        bias=b1_sb[:, 0:1], scale=s12)
            finish_early(h1_sb)
            continue
        if stage == "mmint":
            for m in range(16):
                ps_full = psum.tile([128, NL], f32, name="ps", tag="mm")
                for j in range(KD // 2):
                    for ns in range(4):
                        rhs = xt_int[:, j, ns * 512:(ns + 1) * 512, :]
                        rhs = rhs.transpose([0, 2, 1])
                        nc.tensor.matmul(
                            ps_full[:, ns * 512:(ns + 1) * 512],
                            lhsT=w2_sb[:, m, 2 * j:2 * j + 2, :],
                            rhs=rhs,
                            start=(j == 0), stop=(j + 1 >= KD // 2),
                            perf_mode=perf_mode,
                        )
                if m == 15:
                    nc.scalar.activation(
                        out=h1_sb[:, 0, :], in_=ps_full, func=AF.Relu,
                        bias=b1_sb[:, 0:1], scale=s12)
            finish_early(h1_sb)
            continue
        if stage == "mmplain":
            # L1-shaped work as plain fp8 matmuls (no perf mode): 512 MMs.
            for m in range(16):
                ps_full = psum.tile([128, NL], f32, name="ps", tag="mm")
                for k in range(KD):
                    for ns in range(4):
                        nc.tensor.matmul(
                            ps_full[:, ns * 512:(ns + 1) * 512],
                            lhsT=w2_sb[:, m, k, :],
                            rhs=xt_sb[:, k, ns * 512:(ns + 1) * 512],
                            start=(k == 0), stop=(k + 1 >= KD),
                        )
                if m == 15:
                    nc.scalar.activation(
                        out=h1_sb[:, 0, :], in_=ps_full, func=AF.Relu,
                        bias=b1_sb[:, 0:1], scale=s12)
            finish_early(h1_sb)
            continue
        if stage == "act16":
            ps_full = psum.tile([128, NL], f32, name="ps", tag="mm")
            for ns in range(4):
                nc.tensor.matmul(
                    ps_full[:, ns * 512:(ns + 1) * 512],
                    lhsT=w2_sb[:, 0, 0:2, :],
                    rhs=xt_sb[:, 0:2, ns * 512:(ns + 1) * 512],
                    start=True, stop=True, perf_mode=perf_mode)
            for m in range(16):
                nc.scalar.activation(
                    out=h1_sb[:, m, :], in_=ps_full, func=AF.Relu,
                    bias=b1_sb[:, m:m + 1], scale=s12)
            finish_early(h1_sb)
            continue
        if stage == "dve16":
            ps_full = psum.tile([128, NL], f32, name="ps", tag="mm")
            for ns in range(4):
                nc.tensor.matmul(
                    ps_full[:, ns * 512:(ns + 1) * 512],
                    lhsT=w2_sb[:, 0, 0:2, :],
                    rhs=xt_sb[:, 0:2, ns * 512:(ns + 1) * 512],
                    start=True, stop=True, perf_mode=perf_mode)
            for m in range(16):
                dve_relu(ps_full[:, 0:HB], h1_sb[:, m, 0:HB], b1d_sb, m)
                dve_relu(ps_full[:, HB:NL], h1_sb[:, m, HB:NL], b1d_sb, m)
            finish_early(h1_sb)
            continue

        layer(w1_sb, xt_sb, KD, h1_sb, b1_sb, b1d_sb)
        if stage == "l1":
            finish_early(h1_sb)
            continue
        if stage == "l1x2":
            layer(w1_sb, xt_sb, KD, h1_sb, b1_sb, b1d_sb)
            finish_early(h1_sb)
            continue
        layer(w2_sb, h1_sb, KH, h2_sb, b2_sb, b2d_sb)
        if stage == "l2":
            finish_early(h2_sb)
            continue

        # ---- Layer 3 + log-softmax epilogue in nq column chunks.
        # Per chunk q: DoubleRow matmuls -> ps3; Exp(s3*ps+b3)->bf16;
        # one-hot pick via DVE mult+reduce (s3 rescale carried by the pt
        # matmul lhsT, b3 pick term folded to host); the sum-exp
        # ones-matmul and Ln(accum_out) for chunk q-1 are emitted after
        # chunk q's matmuls so the PE never waits on the Activation
        # engine.
        qw = NL // nq
        expq = [None] * nq
        pick4 = epil.tile([C, nq], f32, name="pick4", tag="pick4")
        lse4 = epil.tile([1, nq], f32, name="lse4", tag="lse4")

        def pse_ln(q):
            pse = psum.tile([128, qw], f32, name="pse", tag="mm")
            for s in range(0, qw, 512):
                nc.tensor.matmul(pse[0:1, s:s + 512], lhsT=ones_bf,
                                 rhs=expq[q][:, s:s + 512],
                                 start=True, stop=True)
            lnscr = epil.tile([1, qw], f32, name="lnscr", tag="lnscr")
            nc.scalar.activation(out=lnscr, in_=pse[0:1, :], func=AF.Ln,
                                 accum_out=lse4[:, q:q + 1])

        for q in range(nq):
            ps3 = psum.tile([128, qw], f32, name="ps3", tag="mm")
            cols = slice(q * qw, (q + 1) * qw)
            for k in range(0, KH, 2):
                for s in range(0, qw, mw):
                    bank_lead = (s * 4) % 2048 == 0  # f32 bank alignment
                    nc.tensor.matmul(
                        ps3[0:CP, s:s + mw],
                        lhsT=w3_sb[:, k:k + 2, :],
                        rhs=h2_sb[:, k:k + 2,
                                  q * qw + s:q * qw + s + mw],
                        start=(k == 0) and bank_lead,
                        stop=(k + 2 >= KH),
                        skip_group_check=not bank_lead,
                        perf_mode=perf_mode,
                    )
            if stage != "e1":
                expq[q] = epil.tile([C, qw], bf16, name=f"exp{q}", tag="expq")
                nc.scalar.activation(out=expq[q], in_=ps3[0:C, :], func=AF.Exp,
                                     bias=b3_sb, scale=s3)
            if stage not in ("e1", "e2"):
                pickscr = epil.tile([C, qw], f32, name="pickscr", tag="pickscr")
                nc.vector.scalar_tensor_tensor(
                    out=pickscr, in0=ps3[0:C, :], scalar=1.0,
                    in1=oh_sb[:, cols], op0=ALU.mult, op1=ALU.mult,
                    accum_out=pick4[:, q:q + 1])
            if stage not in ("e1", "e2", "e3") and q > 0:
                pse_ln(q - 1)
        if stage in ("e1", "e2", "e3"):
            finish_early(h2_sb)
            continue
        pse_ln(nq - 1)

        # total = sum_cq pick4 - sum_q lse4  (b3 pick term added on host)
        pt = psum.tile([128, 8], f32, name="pt", tag="mm")
        nc.tensor.matmul(pt[0:1, 0:nq], lhsT=ones_f, rhs=pick4,
                         start=True, stop=True)
        d = epil.tile([1, nq], f32, name="d", tag="d")
        nc.vector.tensor_tensor(out=d, in0=pt[0:1, 0:nq], in1=lse4,
                                op=ALU.subtract)
        nc.vector.tensor_reduce(out=res_sb, in_=d,
                                axis=mybir.AxisListType.X, op=ALU.add)

    nc.sync.dma_start(out=out, in_=res_sb)


def _build(repeat=1, stage="full", hw_loop=False, lmode="full", nq=NQ, mw=256):
    from contextlib import ExitStack

    import concourse.bacc as bacc
    import concourse.tile as tile
    from concourse import mybir

    f32 = mybir.dt.float32
    fp8 = mybir.dt.float8e4

    nc = bacc.Bacc(
        "TRN2",
        target_bir_lowering=False,
        debug=False,
        enable_asserts=False,
        num_devices=N_CORES,
    )
    aps = {
        "xt": nc.dram_tensor("xt", [128, KD, NL], fp8, kind="ExternalInput").ap(),
        "w1": nc.dram_tensor("w1", [KH, 128, KD, 128], fp8, kind="ExternalInput").ap(),
        "w2": nc.dram_tensor("w2", [KH, 128, KH, 128], fp8, kind="ExternalInput").ap(),
        "w3": nc.dram_tensor("w3", [128, KH, CP], fp8, kind="ExternalInput").ap(),
        "b1": nc.dram_tensor("b1", [128, KH], f32, kind="ExternalInput").ap(),
        "b1d": nc.dram_tensor("b1d", [128, KH], f32, kind="ExternalInput").ap(),
        "b2": nc.dram_tensor("b2", [128, KH], f32, kind="ExternalInput").ap(),
        "b2d": nc.dram_tensor("b2d", [128, KH], f32, kind="ExternalInput").ap(),
        "b3": nc.dram_tensor("b3", [C, 1], f32, kind="ExternalInput").ap(),
        "oh": nc.dram_tensor("oh", [C, NL], f32, kind="ExternalInput").ap(),
        "out": nc.dram_tensor("out", [1, 1], f32, kind="ExternalOutput").ap(),
    }
    with tile.TileContext(nc) as tc:
        with ExitStack() as ctx:
            _emit(tc, ctx, aps, repeat, stage, hw_loop, lmode, nq, mw)
    nc.compile()
    return nc


def _q8(x, s):
    return np.clip(x.astype(np.float32) * s, -240.0, 240.0).astype(E4M3)


def prep_inputs(X, Y, W1, b1, W2, b2, W3, b3):
    W1c = _q8(W1, SW)
    W2c = _q8(W2, SW)
    W3c = _q8(W3, SW)

    w1p = np.ascontiguousarray(W1c.reshape(KH, 128, KD, 128).transpose(0, 3, 2, 1))
    w2p = np.ascontiguousarray(W2c.reshape(KH, 128, KH, 128).transpose(0, 3, 2, 1))
    W3pad = np.zeros((CP, H), dtype=W3c.dtype)
    W3pad[:C] = W3c
    w3p = np.ascontiguousarray(W3pad.reshape(CP, KH, 128).transpose(2, 1, 0))

    b1f = b1.astype(np.float32)
    b2f = b2.astype(np.float32)
    b1p = np.ascontiguousarray((b1f * SH).reshape(KH, 128).T)
    b1dp = np.ascontiguousarray((b1f * SX * SW).reshape(KH, 128).T)
    b2p = np.ascontiguousarray((b2f * SH).reshape(KH, 128).T)
    b2dp = np.ascontiguousarray((b2f * SH * SW).reshape(KH, 128).T)
    b3p = np.ascontiguousarray(b3.astype(np.float32).reshape(C, 1))

    Xb = _q8(X, SX)
    in_maps = []
    for c in range(N_CORES):
        Xc = Xb[c * NL:(c + 1) * NL]
        xtp = np.ascontiguousarray(Xc.reshape(NL, KD, 128).transpose(2, 1, 0))
        Yc = Y[c * NL:(c + 1) * NL]
        ohp = (np.arange(C, dtype=np.int64)[:, None] == Yc[None, :].astype(np.int64))
        ohp = np.ascontiguousarray(ohp.astype(np.float32))
        in_maps.append({
            "xt": xtp, "w1": w1p, "w2": w2p, "w3": w3p,
            "b1": b1p, "b1d": b1dp, "b2": b2p, "b2d": b2dp,
            "b3": b3p, "oh": ohp,
        })
    return in_maps


def log_prior(W1, b1, W2, b2, W3, b3):
    params = (W1, b1, W2, b2, W3, b3)
    d = sum(p.size for p in params)
    sq = sum(float(np.sum(p.astype(np.float64) ** 2)) for p in params)
    return -0.5 * (sq / PRIOR_VAR + d * math.log(2.0 * math.pi * PRIOR_VAR))


def _get_nc(repeat=1, hw_loop=False, stage="full", lmode="full", nq=NQ, mw=256):
    key = (repeat, hw_loop, stage, lmode, nq, mw)
    if key not in _compiled:
        _compiled[key] = _build(repeat, stage=stage, hw_loop=hw_loop,
                                lmode=lmode, nq=nq, mw=mw)
    return _compiled[key]


def run_device(in_maps, repeat=1):
    from concourse.bass_utils import run_bass_kernel_spmd

    nc = _get_nc(repeat)
    res = run_bass_kernel_spmd(nc, in_maps, list(range(N_CORES)))
    return [r["out"][0, 0] for r in res.results]


def kernel(X, Y, W1, b1, W2, b2, W3, b3):
    X = np.asarray(X)
    Y = np.asarray(Y)
    W1 = np.asarray(W1)
    b1 = np.asarray(b1)
    W2 = np.asarray(W2)
    b2 = np.asarray(b2)
    W3 = np.asarray(W3)
    b3 = np.asarray(b3)

    in_maps = prep_inputs(X, Y, W1, b1, W2, b2, W3, b3)
    partials = run_device(in_maps)
    total = float(np.sum(np.asarray(partials, dtype=np.float64)))
    # b3 term of the one-hot pick, folded out of the device kernel
    total += float(np.sum(b3.astype(np.float64)[np.asarray(Y, dtype=np.int64)]))
    total += log_prior(W1, b1, W2, b2, W3, b3)
    return np.array(total, dtype=np.float32)
